# revision 2
# baseline (speedup 1.0000x reference)
"""Trainium2 Bass kernel for AbstractEGCN edge-MLP:
    out[e] = concat(x[src[e]], x[dst[e]]) @ W.T + b      (E=1.6M edges, 100K nodes, 64 feats)

Strategy (8 cores, edge-parallel):
  out[e] = Y1[src[e]] + Y2b[dst[e]]  where  Y1 = x @ W[:, :64].T  and
  Y2b = x @ W[:, 64:].T + b   (node tables, computed on-device per core).

Per core (200K edges):
  Phase 1: compute Y1/Y2b node tables in bf16, write to DRAM. PE transposes
    x chunks (128 nodes), then one fp32 matmul per chunk against a host-packed
    [65, 128] weight block ([W1.T | W2.T] plus a ones-row carrying the bias).
  Phase 2: per 8064-edge block, gather the needed rows with gpsimd dma_gather.
    dma_gather indices are int16, so tables are viewed as [50048, 128] packed
    pairs (node n lives in row n>>1, half n&1) and the gather base is offset
    by 32768 rows so signed int16 covers all 50048 rows. A parity-masked
    copy_predicated selects the right 64-value half; a DVE add produces fp32.

Inputs are sharded/packed on the host; the full [200064, 64] per-core output
is concatenated and truncated to [1.6M, 64] on the host.
"""
import numpy as np
import ml_dtypes
from contextlib import ExitStack

import concourse.bass as bass
import concourse.tile as tile
import concourse.bacc as bacc
from concourse import mybir
from concourse.bass_utils import run_bass_kernel_spmd
from concourse.masks import make_identity

N_NODES = 100000
N_PAD = 100096            # 782 * 128
HID = 64
OUT = 64
N_CORES = 8
E_TOTAL = 1600000
E_CORE = E_TOTAL // N_CORES      # 200000
C_FULL = 63                      # edge columns per gather block
BLK = 128 * C_FULL               # 8064 edges per full block
N_FULL_BLK = 24
C_TAIL = 51
E_CORE_PAD = N_FULL_BLK * BLK + 128 * C_TAIL   # 200064
N_BLK = N_FULL_BLK + 1
IDX_COLS_MAX = (128 * C_FULL + 16) // 16       # 505
BASE_ROW = 32768                 # gather base offset into the packed table
PACKED_ROWS = N_PAD // 2         # 50048


def _block_geom(b):
    C = C_FULL if b < N_FULL_BLK else C_TAIL
    nidx = 128 * C + 16
    return C, nidx, nidx // 16


def build_program():
    nc = bacc.Bacc("TRN2", target_bir_lowering=False, debug=False,
                   num_devices=N_CORES)
    f32, bf16, i16 = mybir.dt.float32, mybir.dt.bfloat16, mybir.dt.int16

    x = nc.dram_tensor("x", [N_PAD, HID], f32, kind="ExternalInput").ap()
    wtb = nc.dram_tensor("wtb", [HID + 1, 2 * OUT], f32, kind="ExternalInput").ap()
    idxs = nc.dram_tensor("idxs", [N_BLK, 128, IDX_COLS_MAX], i16, kind="ExternalInput").ap()
    idxd = nc.dram_tensor("idxd", [N_BLK, 128, IDX_COLS_MAX], i16, kind="ExternalInput").ap()
    pars = nc.dram_tensor("pars", [N_BLK, 128, C_FULL], mybir.dt.uint8, kind="ExternalInput").ap()
    pard = nc.dram_tensor("pard", [N_BLK, 128, C_FULL], mybir.dt.uint8, kind="ExternalInput").ap()
    out = nc.dram_tensor("out", [E_CORE_PAD, OUT], f32, kind="ExternalOutput").ap()

    t1 = nc.dram_tensor("t1", [N_PAD, OUT], bf16).ap()
    t2 = nc.dram_tensor("t2", [N_PAD, OUT], bf16).ap()

    # chunk-major views: [128, 782, 64]; element (p, c, f) = row c*128+p
    xv = x.rearrange("(c p) d -> p c d", p=128)
    t1w = t1.rearrange("(c p) d -> p c d", p=128)
    t2w = t2.rearrange("(c p) d -> p c d", p=128)
    # packed-pair gather views: [50048, 128]
    t1v = t1.rearrange("(r two) d -> r (two d)", two=2)
    t2v = t2.rearrange("(r two) d -> r (two d)", two=2)

    n_chunks = N_PAD // 128                     # 782
    groups = []
    g0 = 0
    while g0 < n_chunks:
        k = min(8, n_chunks - g0)
        groups.append((g0, k))
        g0 += k

    with tile.TileContext(nc) as tc:
        with ExitStack() as ctx:
            cpool = ctx.enter_context(tc.tile_pool(name="const", bufs=1))
            ident = cpool.tile([128, 128], f32)
            make_identity(nc, ident[:])
            wtb_sb = cpool.tile([HID + 1, 2 * OUT], f32)
            nc.sync.dma_start(out=wtb_sb[:, :], in_=wtb[:])

            # ---------------- Phase 1: node tables ----------------
            with ExitStack() as p1:
                pool = p1.enter_context(tc.tile_pool(name="p1", bufs=3))
                psum = p1.enter_context(tc.tile_pool(name="p1ps", bufs=2, space="PSUM"))
                for (g, k) in groups:
                    xt = pool.tile([128, 8, HID], f32, tag="xt")
                    nc.sync.dma_start(out=xt[:, :k, :], in_=xv[:, g:g + k, :])
                    psT = psum.tile([HID, 8, 128], f32, tag="psT")
                    for c in range(k):
                        nc.tensor.transpose(out=psT[:, c, :], in_=xt[:, c, :],
                                            identity=ident[:])
                    xTo = pool.tile([HID + 1, 8, 128], f32, tag="xTo")
                    nc.vector.tensor_copy(out=xTo[0:HID, :k, :], in_=psT[:, :k, :])
                    nc.vector.memset(xTo[HID:HID + 1, :k, :], 1.0)
                    ps2 = psum.tile([128, 8, 2 * OUT], f32, tag="ps2")
                    for c in range(k):
                        nc.tensor.matmul(out=ps2[:, c, :], lhsT=xTo[:, c, :],
                                         rhs=wtb_sb[:, :], start=True, stop=True)
                    yb = pool.tile([128, 8, 2 * OUT], bf16, tag="yb")
                    nc.vector.tensor_copy(out=yb[:, :k, :], in_=ps2[:, :k, :])
                    nc.sync.dma_start(out=t1w[:, g:g + k, :], in_=yb[:, :k, 0:OUT])
                    nc.sync.dma_start(out=t2w[:, g:g + k, :], in_=yb[:, :k, OUT:2 * OUT])

            tc.strict_bb_all_engine_barrier()

            # ---------------- Phase 2: edge gather + combine ----------------
            with ExitStack() as p2:
                pool = p2.enter_context(tc.tile_pool(name="p2", bufs=2))
                ipool = p2.enter_context(tc.tile_pool(name="p2i", bufs=3))
                for b in range(N_BLK):
                    C, nidx, cols = _block_geom(b)
                    isrc = ipool.tile([128, IDX_COLS_MAX], i16, tag="is")
                    nc.sync.dma_start(out=isrc[:, :cols], in_=idxs[b, :, :cols])
                    idst = ipool.tile([128, IDX_COLS_MAX], i16, tag="id")
                    nc.sync.dma_start(out=idst[:, :cols], in_=idxd[b, :, :cols])
                    pm_s = ipool.tile([128, C_FULL], mybir.dt.uint8, tag="pms")
                    nc.sync.dma_start(out=pm_s[:, :C], in_=pars[b, :, :C])
                    pm_d = ipool.tile([128, C_FULL], mybir.dt.uint8, tag="pmd")
                    nc.sync.dma_start(out=pm_d[:, :C], in_=pard[b, :, :C])

                    ga = pool.tile([128, C_FULL + 1, 2 * OUT], bf16, tag="ga")
                    nc.gpsimd.dma_gather(
                        out_ap=ga[:, :C + 1, :],
                        in_ap=t1v[BASE_ROW:PACKED_ROWS],
                        idxs_ap=isrc[:, :cols],
                        num_idxs=nidx,
                        num_idxs_reg=nidx,
                        elem_size=2 * OUT,
                        single_packet=False,
                    )
                    gb = pool.tile([128, C_FULL + 1, 2 * OUT], bf16, tag="gb")
                    nc.gpsimd.dma_gather(
                        out_ap=gb[:, :C + 1, :],
                        in_ap=t2v[BASE_ROW:PACKED_ROWS],
                        idxs_ap=idst[:, :cols],
                        num_idxs=nidx,
                        num_idxs_reg=nidx,
                        elem_size=2 * OUT,
                        single_packet=False,
                    )

                    se = pool.tile([128, C_FULL, OUT], bf16, tag="se")
                    nc.any.tensor_copy(out=se[:, :C, :], in_=ga[:, :C, 0:OUT])
                    nc.vector.copy_predicated(
                        out=se[:, :C, :],
                        mask=pm_s[:, :C].to_broadcast([128, C, OUT]),
                        data=ga[:, :C, OUT:2 * OUT],
                    )
                    de = pool.tile([128, C_FULL, OUT], bf16, tag="de")
                    nc.any.tensor_copy(out=de[:, :C, :], in_=gb[:, :C, 0:OUT])
                    nc.vector.copy_predicated(
                        out=de[:, :C, :],
                        mask=pm_d[:, :C].to_broadcast([128, C, OUT]),
                        data=gb[:, :C, OUT:2 * OUT],
                    )
                    ob = pool.tile([128, C_FULL, OUT], f32, tag="ob")
                    nc.vector.tensor_add(out=ob[:, :C, :], in0=se[:, :C, :],
                                         in1=de[:, :C, :])
                    ov = out[b * BLK: b * BLK + 128 * C].rearrange(
                        "(p c) d -> p c d", p=128)
                    nc.sync.dma_start(out=ov[:, :, :], in_=ob[:, :C, :])
    nc.compile()
    return nc


def _host_prep_core(src, dst):
    """src/dst: [E_CORE] int64 node ids -> per-core idx/parity arrays."""
    i16 = np.int16
    srcp = np.zeros(E_CORE_PAD, dtype=np.int64)
    dstp = np.zeros(E_CORE_PAD, dtype=np.int64)
    srcp[:E_CORE] = src
    dstp[:E_CORE] = dst
    idx_s = np.zeros((N_BLK, 128, IDX_COLS_MAX), dtype=i16)
    idx_d = np.zeros((N_BLK, 128, IDX_COLS_MAX), dtype=i16)
    par_s = np.zeros((N_BLK, 128, C_FULL), dtype=np.uint8)
    par_d = np.zeros((N_BLK, 128, C_FULL), dtype=np.uint8)
    e0 = 0
    for b in range(N_BLK):
        C, nidx, cols = _block_geom(b)
        for nodes, idx_arr, par_arr in ((srcp, idx_s, par_s), (dstp, idx_d, par_d)):
            blk = nodes[e0:e0 + 128 * C].reshape(128, C)   # [p, c] = edge p*C+c
            slots = np.zeros(nidx, dtype=i16)
            slots[:128 * C] = ((blk >> 1) - BASE_ROW).astype(i16).T.ravel()
            # trailing pad slots stay 0 (>= 0, disarms ucode tail trimming)
            w = slots.reshape(cols, 16).T                   # slot j -> (j%16, j//16)
            idx_arr[b, :, :cols] = np.tile(w, (8, 1))
            par_arr[b, :, :C] = (blk & 1).astype(np.uint8)
        e0 += 128 * C
    return idx_s, idx_d, par_s, par_d


_NC_CACHE = []


def kernel(x, edge_index, W, b):
    x = np.asarray(x, dtype=np.float32)
    edge_index = np.asarray(edge_index)
    W = np.asarray(W, dtype=np.float32)
    b = np.asarray(b, dtype=np.float32)

    if not _NC_CACHE:
        _NC_CACHE.append(build_program())
    nc = _NC_CACHE[0]

    x_pad = np.zeros((N_PAD, HID), dtype=np.float32)
    x_pad[:N_NODES] = x
    wtb = np.zeros((HID + 1, 2 * OUT), dtype=np.float32)
    wtb[0:HID, 0:OUT] = W[:, 0:HID].T
    wtb[0:HID, OUT:2 * OUT] = W[:, HID:2 * HID].T
    wtb[HID, OUT:2 * OUT] = b

    src_all = np.asarray(edge_index[0], dtype=np.int64)
    dst_all = np.asarray(edge_index[1], dtype=np.int64)

    in_maps = []
    for k in range(N_CORES):
        e0 = k * E_CORE
        idx_s, idx_d, par_s, par_d = _host_prep_core(
            src_all[e0:e0 + E_CORE], dst_all[e0:e0 + E_CORE])
        in_maps.append({
            "x": x_pad, "wtb": wtb,
            "idxs": idx_s, "idxd": idx_d,
            "pars": par_s, "pard": par_d,
        })

    res = run_bass_kernel_spmd(nc, in_maps, list(range(N_CORES)))
    outs = [np.asarray(res.results[k]["out"][:E_CORE]) for k in range(N_CORES)]
    return np.concatenate(outs, axis=0)


# revision 3
# speedup vs baseline: 607.5718x; 607.5718x over previous
"""Trainium2 Bass kernel for AbstractEGCN edge-MLP:
    out[e] = concat(x[src[e]], x[dst[e]]) @ W.T + b      (E=1.6M edges, 100K nodes, 64 feats)

Strategy (8 cores, edge-parallel):
  out[e] = Y1[src[e]] + Y2b[dst[e]]  where  Y1 = x @ W[:, :64].T  and
  Y2b = x @ W[:, 64:].T + b   (node tables, computed on-device per core).

Per core (200K edges):
  Phase 1: compute Y1/Y2b node tables in bf16, write to DRAM. PE transposes
    x chunks (128 nodes), then one fp32 matmul per chunk against a host-packed
    [65, 128] weight block ([W1.T | W2.T] plus a ones-row carrying the bias).
  Phase 2: per 8064-edge block, gather the needed rows with gpsimd dma_gather.
    dma_gather indices are int16, so tables are viewed as [50048, 128] packed
    pairs (node n lives in row n>>1, half n&1) and the gather base is offset
    by 32768 rows so signed int16 covers all 50048 rows. A parity-masked
    copy_predicated selects the right 64-value half; a DVE add produces fp32.

Inputs are sharded/packed on the host; the full [200064, 64] per-core output
is concatenated and truncated to [1.6M, 64] on the host.
"""
import numpy as np
import ml_dtypes
from contextlib import ExitStack

import concourse.bass as bass
import concourse.tile as tile
import concourse.bacc as bacc
from concourse import mybir
from concourse.bass_utils import run_bass_kernel_spmd
from concourse.masks import make_identity

N_NODES = 100000
N_PAD = 100096            # 782 * 128
HID = 64
OUT = 64
N_CORES = 8
E_TOTAL = 1600000
E_CORE = E_TOTAL // N_CORES      # 200000
C_FULL = 63                      # edge columns per gather block
BLK = 128 * C_FULL               # 8064 edges per full block
N_FULL_BLK = 24
C_TAIL = 51
E_CORE_PAD = N_FULL_BLK * BLK + 128 * C_TAIL   # 200064
N_BLK = N_FULL_BLK + 1
IDX_COLS_MAX = (128 * C_FULL + 16) // 16       # 505
BASE_ROW = 32768                 # gather base offset into the packed table
PACKED_ROWS = N_PAD // 2         # 50048


def _block_geom(b):
    C = C_FULL if b < N_FULL_BLK else C_TAIL
    nidx = 128 * C + 16
    return C, nidx, nidx // 16


def build_program():
    nc = bacc.Bacc("TRN2", target_bir_lowering=False, debug=False,
                   num_devices=N_CORES)
    f32, bf16, i16 = mybir.dt.float32, mybir.dt.bfloat16, mybir.dt.int16

    x = nc.dram_tensor("x", [N_PAD, HID], f32, kind="ExternalInput").ap()
    wtb = nc.dram_tensor("wtb", [HID + 1, 2 * OUT], f32, kind="ExternalInput").ap()
    idxs = nc.dram_tensor("idxs", [N_BLK, 128, IDX_COLS_MAX], i16, kind="ExternalInput").ap()
    idxd = nc.dram_tensor("idxd", [N_BLK, 128, IDX_COLS_MAX], i16, kind="ExternalInput").ap()
    pars = nc.dram_tensor("pars", [N_BLK, 128, C_FULL], mybir.dt.uint8, kind="ExternalInput").ap()
    pard = nc.dram_tensor("pard", [N_BLK, 128, C_FULL], mybir.dt.uint8, kind="ExternalInput").ap()
    out = nc.dram_tensor("out", [E_CORE_PAD, OUT], f32, kind="ExternalOutput").ap()

    t1 = nc.dram_tensor("t1", [N_PAD, OUT], bf16).ap()
    t2 = nc.dram_tensor("t2", [N_PAD, OUT], bf16).ap()

    # chunk-major views: [128, 782, 64]; element (p, c, f) = row c*128+p
    xv = x.rearrange("(c p) d -> p c d", p=128)
    t1w = t1.rearrange("(c p) d -> p c d", p=128)
    t2w = t2.rearrange("(c p) d -> p c d", p=128)
    # packed-pair gather views: [50048, 128]
    t1v = t1.rearrange("(r two) d -> r (two d)", two=2)
    t2v = t2.rearrange("(r two) d -> r (two d)", two=2)

    n_chunks = N_PAD // 128                     # 782
    groups = []
    g0 = 0
    while g0 < n_chunks:
        k = min(8, n_chunks - g0)
        groups.append((g0, k))
        g0 += k

    with tile.TileContext(nc) as tc:
        with ExitStack() as ctx:
            cpool = ctx.enter_context(tc.tile_pool(name="const", bufs=1))
            ident = cpool.tile([128, 128], f32)
            make_identity(nc, ident[:])
            wtb_sb = cpool.tile([HID + 1, 2 * OUT], f32)
            nc.sync.dma_start(out=wtb_sb[:, :], in_=wtb[:])

            # ---------------- Phase 1: node tables ----------------
            with ExitStack() as p1:
                pool = p1.enter_context(tc.tile_pool(name="p1", bufs=3))
                psum = p1.enter_context(tc.tile_pool(name="p1ps", bufs=2, space="PSUM"))
                for (g, k) in groups:
                    xt = pool.tile([128, 8, HID], f32, tag="xt")
                    nc.sync.dma_start(out=xt[:, :k, :], in_=xv[:, g:g + k, :])
                    psT = psum.tile([HID, 8, 128], f32, tag="psT")
                    for c in range(k):
                        nc.tensor.transpose(out=psT[:, c, :], in_=xt[:, c, :],
                                            identity=ident[:])
                    xTo = pool.tile([HID + 1, 8, 128], f32, tag="xTo")
                    nc.vector.tensor_copy(out=xTo[0:HID, :k, :], in_=psT[:, :k, :])
                    nc.vector.memset(xTo[HID:HID + 1, :k, :], 1.0)
                    ps2 = psum.tile([128, 8, 2 * OUT], f32, tag="ps2")
                    for c in range(k):
                        nc.tensor.matmul(out=ps2[:, c, :], lhsT=xTo[:, c, :],
                                         rhs=wtb_sb[:, :], start=True, stop=True)
                    yb = pool.tile([128, 8, 2 * OUT], bf16, tag="yb")
                    nc.vector.tensor_copy(out=yb[:, :k, :], in_=ps2[:, :k, :])
                    nc.sync.dma_start(out=t1w[:, g:g + k, :], in_=yb[:, :k, 0:OUT])
                    nc.sync.dma_start(out=t2w[:, g:g + k, :], in_=yb[:, :k, OUT:2 * OUT])

            tc.strict_bb_all_engine_barrier()

            # ---------------- Phase 2: edge gather + combine ----------------
            with ExitStack() as p2:
                pool = p2.enter_context(tc.tile_pool(name="p2", bufs=2))
                ipool = p2.enter_context(tc.tile_pool(name="p2i", bufs=3))
                for b in range(N_BLK):
                    C, nidx, cols = _block_geom(b)
                    isrc = ipool.tile([128, IDX_COLS_MAX], i16, tag="is")
                    nc.sync.dma_start(out=isrc[:, :cols], in_=idxs[b, :, :cols])
                    idst = ipool.tile([128, IDX_COLS_MAX], i16, tag="id")
                    nc.sync.dma_start(out=idst[:, :cols], in_=idxd[b, :, :cols])
                    pm_s = ipool.tile([128, C_FULL], mybir.dt.uint8, tag="pms")
                    nc.sync.dma_start(out=pm_s[:, :C], in_=pars[b, :, :C])
                    pm_d = ipool.tile([128, C_FULL], mybir.dt.uint8, tag="pmd")
                    nc.sync.dma_start(out=pm_d[:, :C], in_=pard[b, :, :C])

                    ga = pool.tile([128, C_FULL + 1, 2 * OUT], bf16, tag="ga")
                    nc.gpsimd.dma_gather(
                        out_ap=ga[:, :C + 1, :],
                        in_ap=t1v[BASE_ROW:PACKED_ROWS],
                        idxs_ap=isrc[:, :cols],
                        num_idxs=nidx,
                        num_idxs_reg=nidx,
                        elem_size=2 * OUT,
                        single_packet=False,
                    )
                    gb = pool.tile([128, C_FULL + 1, 2 * OUT], bf16, tag="gb")
                    nc.gpsimd.dma_gather(
                        out_ap=gb[:, :C + 1, :],
                        in_ap=t2v[BASE_ROW:PACKED_ROWS],
                        idxs_ap=idst[:, :cols],
                        num_idxs=nidx,
                        num_idxs_reg=nidx,
                        elem_size=2 * OUT,
                        single_packet=False,
                    )

                    se = pool.tile([128, C_FULL, OUT], bf16, tag="se")
                    nc.any.tensor_copy(out=se[:, :C, :], in_=ga[:, :C, 0:OUT])
                    nc.vector.copy_predicated(
                        out=se[:, :C, :],
                        mask=pm_s[:, :C].to_broadcast([128, C, OUT]),
                        data=ga[:, :C, OUT:2 * OUT],
                    )
                    de = pool.tile([128, C_FULL, OUT], bf16, tag="de")
                    nc.any.tensor_copy(out=de[:, :C, :], in_=gb[:, :C, 0:OUT])
                    nc.vector.copy_predicated(
                        out=de[:, :C, :],
                        mask=pm_d[:, :C].to_broadcast([128, C, OUT]),
                        data=gb[:, :C, OUT:2 * OUT],
                    )
                    ob = pool.tile([128, C_FULL, OUT], f32, tag="ob")
                    nc.vector.tensor_add(out=ob[:, :C, :], in0=se[:, :C, :],
                                         in1=de[:, :C, :])
                    ov = out[b * BLK: b * BLK + 128 * C].rearrange(
                        "(p c) d -> p c d", p=128)
                    nc.sync.dma_start(out=ov[:, :, :], in_=ob[:, :C, :])
    nc.compile()
    return nc


def _host_prep_core(src, dst):
    """src/dst: [E_CORE] int64 node ids -> per-core idx/parity arrays."""
    i16 = np.int16
    srcp = np.zeros(E_CORE_PAD, dtype=np.int64)
    dstp = np.zeros(E_CORE_PAD, dtype=np.int64)
    srcp[:E_CORE] = src
    dstp[:E_CORE] = dst
    idx_s = np.zeros((N_BLK, 128, IDX_COLS_MAX), dtype=i16)
    idx_d = np.zeros((N_BLK, 128, IDX_COLS_MAX), dtype=i16)
    par_s = np.zeros((N_BLK, 128, C_FULL), dtype=np.uint8)
    par_d = np.zeros((N_BLK, 128, C_FULL), dtype=np.uint8)
    e0 = 0
    for b in range(N_BLK):
        C, nidx, cols = _block_geom(b)
        for nodes, idx_arr, par_arr in ((srcp, idx_s, par_s), (dstp, idx_d, par_d)):
            blk = nodes[e0:e0 + 128 * C].reshape(128, C)   # [p, c] = edge p*C+c
            slots = np.zeros(nidx, dtype=i16)
            slots[:128 * C] = ((blk >> 1) - BASE_ROW).astype(i16).T.ravel()
            # trailing pad slots stay 0 (>= 0, disarms ucode tail trimming)
            w = slots.reshape(cols, 16).T                   # slot j -> (j%16, j//16)
            idx_arr[b, :, :cols] = np.tile(w, (8, 1))
            par_arr[b, :, :C] = (blk & 1).astype(np.uint8)
        e0 += 128 * C
    return idx_s, idx_d, par_s, par_d


_EXEC_CACHE = {}

# inputs replicated across cores (same array on every core)
_REPLICATED = {"x", "wtb"}


def _get_exec():
    """Build + compile the program once and return a cached jitted runner.

    Reimplements the tail of concourse.bass2jax.run_bass_via_pjrt but caches
    the jitted callable (fresh jax.jit closures there force a full retrace
    per call), replicates x/wtb instead of shipping 8 copies, and skips
    output-buffer donation (every output element is written by the kernel)
    so timing loops can reuse device-resident zero buffers.
    """
    if _EXEC_CACHE:
        return _EXEC_CACHE["run"]

    import jax
    from jax.experimental.shard_map import shard_map
    from jax.sharding import Mesh, PartitionSpec
    from concourse import bass2jax, mybir as mb

    nc = build_program()
    bass2jax.install_neuronx_cc_hook()

    partition_name = nc.partition_id_tensor.name if nc.partition_id_tensor else None
    in_names, out_names, out_avals = [], [], []
    for alloc in nc.m.functions[0].allocations:
        if not isinstance(alloc, mb.MemoryLocationSet):
            continue
        name = alloc.memorylocations[0].name
        if alloc.kind == "ExternalInput":
            if name != partition_name:
                in_names.append(name)
        elif alloc.kind == "ExternalOutput":
            out_names.append(name)
            out_avals.append(jax.core.ShapedArray(
                tuple(alloc.tensor_shape), mb.dt.np(alloc.dtype)))
    n_params = len(in_names)
    all_names = in_names + out_names
    if partition_name is not None:
        all_names.append(partition_name)

    def _body(*args):
        operands = list(args)
        if partition_name is not None:
            operands.append(bass2jax.partition_id_tensor())
        outs = bass2jax._bass_exec_p.bind(
            *operands,
            out_avals=tuple(out_avals),
            in_names=tuple(all_names),
            out_names=tuple(out_names),
            lowering_input_output_aliases=(),
            sim_require_finite=True,
            sim_require_nnan=True,
            nc=nc,
        )
        return tuple(outs)

    devices = jax.devices()[:N_CORES]
    mesh = Mesh(np.asarray(devices), ("core",))
    in_specs = tuple(
        PartitionSpec(None) if name in _REPLICATED else PartitionSpec("core")
        for name in in_names
    ) + (PartitionSpec("core"),) * len(out_names)
    out_specs = (PartitionSpec("core"),) * len(out_names)
    sharded = jax.jit(shard_map(
        _body, mesh=mesh, in_specs=in_specs, out_specs=out_specs, check_rep=False))

    zero_outs = [np.zeros((N_CORES * a.shape[0], *a.shape[1:]), a.dtype)
                 for a in out_avals]

    def run(per_core_maps):
        args = []
        for name in in_names:
            if name in _REPLICATED:
                args.append(per_core_maps[0][name])
            else:
                args.append(np.concatenate(
                    [per_core_maps[c][name] for c in range(N_CORES)], axis=0))
        outs = sharded(*args, *zero_outs)
        return [
            {name: np.asarray(outs[i]).reshape(N_CORES, *out_avals[i].shape)[c]
             for i, name in enumerate(out_names)}
            for c in range(N_CORES)
        ]

    _EXEC_CACHE["run"] = run
    _EXEC_CACHE["internals"] = (sharded, in_names, out_names, out_avals, zero_outs)
    return run


def host_prep(x, edge_index, W, b):
    x = np.asarray(x, dtype=np.float32)
    edge_index = np.asarray(edge_index)
    W = np.asarray(W, dtype=np.float32)
    b = np.asarray(b, dtype=np.float32)

    x_pad = np.zeros((N_PAD, HID), dtype=np.float32)
    x_pad[:N_NODES] = x
    wtb = np.zeros((HID + 1, 2 * OUT), dtype=np.float32)
    wtb[0:HID, 0:OUT] = W[:, 0:HID].T
    wtb[0:HID, OUT:2 * OUT] = W[:, HID:2 * HID].T
    wtb[HID, OUT:2 * OUT] = b

    src_all = np.asarray(edge_index[0], dtype=np.int64)
    dst_all = np.asarray(edge_index[1], dtype=np.int64)

    in_maps = []
    for k in range(N_CORES):
        e0 = k * E_CORE
        idx_s, idx_d, par_s, par_d = _host_prep_core(
            src_all[e0:e0 + E_CORE], dst_all[e0:e0 + E_CORE])
        in_maps.append({
            "x": x_pad, "wtb": wtb,
            "idxs": idx_s, "idxd": idx_d,
            "pars": par_s, "pard": par_d,
        })
    return in_maps


def kernel(x, edge_index, W, b):
    run = _get_exec()
    in_maps = host_prep(x, edge_index, W, b)
    results = run(in_maps)
    outs = [np.asarray(results[k]["out"][:E_CORE]) for k in range(N_CORES)]
    return np.concatenate(outs, axis=0)
